# revision 15
# baseline (speedup 1.0000x reference)
"""DialogueGCN forward pass as a distributed Bass/Tile kernel on 8 TRN2 NeuronCores.

Math (reference): Bahdanau attention over utterance pairs -> per-edge softmax
weights; RGCN with per-relation weights W_rel[etype] + root term; GraphConv.

Key structural facts exploited:
  * etype = 2*(sp[i]*48 + sp[j]) + (i>=j) with speaker in {0,1} -> at most 8
    distinct relation types out of R=4608 are live. Only those 8 slices of the
    1.2GB W_rel are ever touched (host routes them to the devices).
  * The graph is fully connected, so the GraphConv neighbor sum is the same
    vector for every node: sum_i h_i.
  * agg = sum_r (attn*mask_r)^T (x @ W_r): 8 masked matmuls instead of a
    2304-edge gather/scatter.

Sharding: the RGCN/GraphConv hidden dim H=256 is split 8 ways (32 columns per
core); every core computes the full [48,48] attention (cheap, avoids a second
collective), its h-slice of the RGCN, then one AllGather of the [32,48] slices
rebuilds h^T [256,48] on every core, and each core finishes its g-slice of the
GraphConv output. Host concatenates the 8 [32,48] outputs and transposes.
"""
import numpy as np

L = 48
D = 256
H = 256
G = 256
A = 128
N_CORES = 8
HS = H // N_CORES  # 32 columns of h (and of the output) per core
NREL = 8

_compiled = None


def _emit_body(nc, mybir, pool, psum, dram, d, rep, collective, n_cores):
    """Emit one full forward pass. `d` maps dram-tensor names to handles."""
    dt = mybir.dt.float32
    u = f"_{rep}"

    # ---- three packed input DMAs, on three different engine queues ----
    apk = pool.tile([128, 2, 304], dt, name=f"apk{u}", tag="apk")
    rpk = pool.tile([128, 2, 352], dt, name=f"rpk{u}", tag="rpk")
    spk = pool.tile([128, 387], dt, name=f"spk{u}", tag="spk")
    nc.sync.dma_start(apk[:], d["apack"].ap().rearrange("t p f -> p t f"))
    nc.scalar.dma_start(spk[:], d["spack"].ap())
    nc.gpsimd.dma_start(rpk[:], d["rpack"].ap().rearrange("t p f -> p t f"))

    def xt(t):
        return apk[:, t, 0:L]
    def wqs(t):
        return apk[:, t, L:L + A]
    def wks(t):
        return apk[:, t, L + A:L + 2 * A]
    def wr(t):
        return rpk[:, t, 0:NREL * HS]
    def wro(t):
        return rpk[:, t, NREL * HS:NREL * HS + HS]
    def wsl(t):
        return rpk[:, t, NREL * HS + HS:NREL * HS + 2 * HS]
    def wn(t):
        return rpk[:, t, NREL * HS + 2 * HS:NREL * HS + 3 * HS]
    vv = spk[:, 0:1]
    maskt = spk[0:L, 1:1 + NREL * L].rearrange("i (r j) -> i r j", r=NREL)
    brg = spk[0:HS, 1 + NREL * L:2 + NREL * L]
    bgc = spk[0:HS, 2 + NREL * L:3 + NREL * L]

    # f32r (TF32-like) matmul operands: 4x PE rate on wide outputs, ~1e-3
    # matmul precision -- well inside the output tolerance here.
    f32r = mybir.dt.float32r
    # ---- Bahdanau attention: scores[i,j] = v . tanh(qT[:,i]+kT[:,j]) ----
    qT_ps = psum.tile([128, L], dt, name=f"qT_ps{u}", tag="attn_ps")
    for t in range(2):
        nc.tensor.matmul(qT_ps[:], wqs(t), xt(t), start=(t == 0), stop=(t == 1))
    qTs = pool.tile([128, L, 1], dt, name=f"qTs{u}", tag="qTs")
    nc.vector.tensor_copy(qTs[:, :, 0], qT_ps[:])
    kT_ps = psum.tile([128, L], dt, name=f"kT_ps{u}", tag="attn_ps")
    for t in range(2):
        nc.tensor.matmul(kT_ps[:], wks(t), xt(t), start=(t == 0), stop=(t == 1))
    kTs = pool.tile([128, 1, L], dt, name=f"kTs{u}", tag="kTs")
    nc.vector.tensor_copy(kTs[:, 0, :], kT_ps[:])

    # broadcast add + tanh + v-matmul, in 2 halves so DVE/ACT/PE pipeline
    HL = L // 2  # 24 query rows per half
    bigT = pool.tile([128, L, L], dt, name=f"bigT{u}", tag="bigT")
    # tanh output + v are written as float32r so the score matmuls can run
    # the 4x-rate f32r PE path (producers must round to f32r per verifier)
    tanhT = pool.tile([128, L * L], f32r, name=f"tanhT{u}", tag="tanhT")
    vvr = pool.tile([128, 1], f32r, name=f"vvr{u}", tag="vvr")
    nc.vector.tensor_copy(vvr[:], vv)
    scores_ps = psum.tile([1, L * L], dt, name=f"scores_ps{u}", tag="attn_big")
    for h in range(2):
        isl = slice(h * HL, (h + 1) * HL)
        csl = slice(h * HL * L, (h + 1) * HL * L)
        nc.vector.tensor_tensor(
            bigT[:, isl, :],
            qTs[:, isl, :].broadcast_to([128, HL, L]),
            kTs[:].broadcast_to([128, HL, L]),
            op=mybir.AluOpType.add,
        )
        nc.scalar.activation(tanhT[:, csl],
                             bigT[:, isl, :].rearrange("p i j -> p (i j)"),
                             mybir.ActivationFunctionType.Tanh)
        # matmul outputs must stay inside one PSUM bank: chunk on global
        # 512-aligned boundaries; emit the chunks fully covered by the tanh
        # ready so far (half 0: [0:1024]; half 1: the rest).
        lo = 1024 if h == 1 else 0
        hi_lim = HL * L * (h + 1) if h == 0 else L * L
        for k in range(lo, hi_lim - 511 if h == 0 else hi_lim, 512):
            hi = min(k + 512, L * L)
            nc.tensor.matmul(scores_ps[:, k:hi], vvr[:], tanhT[:, k:hi],
                             start=True, stop=True)
    # PSUM -> SBUF [1, 2304]: single-partition copy, split across DVE + ACT
    scores_row = pool.tile([1, L * L], dt, name=f"scores_row{u}", tag="scores_row")
    nc.vector.tensor_copy(scores_row[:, :HL * L], scores_ps[:, :HL * L])
    nc.scalar.copy(scores_row[:, HL * L:], scores_ps[:, HL * L:])
    # reshape [1, 2304] -> [48, 48] has to cross partitions: DRAM bounce
    scores_dram = dram.tile([1, L * L], dt, name=f"scores_dram{u}", tag="scores_dram")
    nc.sync.dma_start(scores_dram[:], scores_row[:])
    scores = pool.tile([L, L], dt, name=f"scores{u}", tag="scores")
    nc.sync.dma_start(scores[:], scores_dram[:].rearrange("p (i j) -> (p i) j", i=L))

    # ---- softmax over j (scores bounded by sum|v| ~ 9: no max pass) ----
    expS = pool.tile([L, L], dt, name=f"expS{u}", tag="expS")
    rowsum = pool.tile([L, 1], dt, name=f"rowsum{u}", tag="rowsum")
    nc.scalar.activation(expS[:], scores[:], mybir.ActivationFunctionType.Exp,
                         accum_out=rowsum[:])
    recip = pool.tile([L, 1], dt, name=f"recip{u}", tag="recip")
    nc.vector.reciprocal(recip[:], rowsum[:])
    # A_r[i,j] = (exp * 1/rowsum) * mask_r, all 8 relations in one op
    attnW = pool.tile([L, NREL, L], dt, name=f"attnW{u}", tag="attnW")
    nc.vector.scalar_tensor_tensor(
        attnW[:],
        expS[:].rearrange("i (o j) -> i o j", o=1).broadcast_to([L, NREL, L]),
        recip[:],
        maskt,
        op0=mybir.AluOpType.mult,
        op1=mybir.AluOpType.mult,
    )

    # ---- RGCN (h-slice): Yall[i, r*32+h'] = (x @ W_r[:, hsl]) ----
    yall_ps = psum.tile([L, NREL * HS], dt, name=f"yall_ps{u}", tag="mm_ps")
    for t in range(2):
        nc.tensor.matmul(yall_ps[:], xt(t), wr(t), start=(t == 0), stop=(t == 1))
    yall = pool.tile([L, NREL * HS], dt, name=f"yall{u}", tag="yall")
    nc.vector.tensor_copy(yall[:], yall_ps[:])

    # hT_slice[h', j] = sum_r sum_i Y_r[i,h'] A_r[i,j] + (x@W_root)^T + b
    h_ps = psum.tile([HS, L], dt, name=f"h_ps{u}", tag="mm_ps")
    for r in range(NREL):
        nc.tensor.matmul(h_ps[:], yall[:, r * HS:(r + 1) * HS], attnW[:, r, :],
                         start=(r == 0), stop=False)
    for t in range(2):
        nc.tensor.matmul(h_ps[:], wro(t), xt(t), start=False, stop=(t == 1))
    hTs = pool.tile([HS, L], dt, name=f"hTs{u}", tag="hTs")
    nc.vector.tensor_scalar_add(hTs[:], h_ps[:], brg)

    # ---- AllGather h-slices -> full hT [256, 48] on every core ----
    ag_in = dram.tile([HS, L], dt, name=f"ag_in{u}", tag="ag_in")
    ag_out = dram.tile([H, L], dt, name=f"ag_out{u}", tag="ag_out")
    nc.sync.dma_start(ag_in[:], hTs[:])
    if collective:
        nc.gpsimd.collective_compute(
            "AllGather",
            mybir.AluOpType.bypass,
            replica_groups=[list(range(n_cores))],
            ins=[ag_in.opt()],
            outs=[ag_out.opt()],
        )
    else:
        # single-core stand-in for TimelineSim: replicate the slice 8x
        agw = ag_out[:].rearrange("(c p) f -> c p f", p=HS)
        for c in range(N_CORES):
            nc.sync.dma_start(agw[c], ag_in[:])
    hfull = pool.tile([128, 2, L], dt, name=f"hfull{u}", tag="hfull")
    nc.sync.dma_start(hfull[:], ag_out[:].rearrange("(t p) f -> p t f", p=128))

    # ---- GraphConv (g-slice): out^T = W_self^T hT + (W_nbr^T s + b) ----
    sT = pool.tile([128, 2], dt, name=f"sT{u}", tag="sT")
    for t in range(2):
        nc.vector.reduce_sum(sT[:, t:t + 1], hfull[:, t, :],
                             axis=mybir.AxisListType.X)
    nb_ps = psum.tile([HS, 1], dt, name=f"nb_ps{u}", tag="nb_ps")
    for t in range(2):
        nc.tensor.matmul(nb_ps[:], wn(t), sT[:, t:t + 1],
                         start=(t == 0), stop=(t == 1))
    nbs = pool.tile([HS, 1], dt, name=f"nbs{u}", tag="nbs")
    nc.vector.tensor_scalar_add(nbs[:], nb_ps[:], bgc)

    out_ps = psum.tile([HS, L], dt, name=f"out_ps{u}", tag="mm_ps")
    for t in range(2):
        nc.tensor.matmul(out_ps[:], wsl(t), hfull[:, t, :],
                         start=(t == 0), stop=(t == 1))
    outs = pool.tile([HS, L], dt, name=f"outs{u}", tag="outs")
    nc.vector.tensor_scalar_add(outs[:], out_ps[:], nbs)
    nc.sync.dma_start(d["yout"].ap(), outs[:])


def build_program(n_cores=N_CORES, collective=True, repeat=1):
    """Build + schedule + compile the Bass program."""
    import concourse.bacc as bacc
    import concourse.mybir as mybir
    import concourse.tile as tile

    dt = mybir.dt.float32
    nc = bacc.Bacc("TRN2", debug=False, num_devices=n_cores)

    d = {}
    d["apack"] = nc.dram_tensor("apack", [2, 128, 304], dt, kind="ExternalInput")
    d["rpack"] = nc.dram_tensor("rpack", [2, 128, 352], dt, kind="ExternalInput")
    d["spack"] = nc.dram_tensor("spack", [128, 387], dt, kind="ExternalInput")
    d["yout"] = nc.dram_tensor("yout", [HS, L], dt, kind="ExternalOutput")

    with tile.TileContext(nc) as tc:
        with (
            tc.tile_pool(name="sbuf", bufs=1) as pool,
            tc.tile_pool(name="psum", bufs=1, space="PSUM") as psum,
            tc.tile_pool(name="dram", bufs=1, space="DRAM") as dram,
        ):
            for rep in range(repeat):
                _emit_body(nc, mybir, pool, psum, dram, d, rep, collective, n_cores)

    nc.compile()
    return nc


def _prepare_in_maps(global_features, speaker, Wq, Wk, v, W_rel, W_root, b_rgcn,
                     W_nbr, W_self, b_gcn):
    """Host-side routing: pick the <=8 live relation slices, build masks, pack
    per-core shards (h-slice of RGCN weights, g-slice of GraphConv weights)."""
    f32 = np.float32
    x = np.ascontiguousarray(global_features, dtype=f32)
    sp = np.asarray(speaker).astype(np.int64)
    n = L

    ii, jj = np.meshgrid(np.arange(n), np.arange(n), indexing="ij")
    direction = (ii >= jj).astype(np.int64)
    et = 2 * (sp[ii] * n + sp[jj]) + direction  # [48, 48] edge-type grid

    rel_ids = np.unique(et)
    assert len(rel_ids) <= NREL, f"{len(rel_ids)} live relations > {NREL}"
    masks = np.zeros((NREL, n, n), dtype=f32)
    rel_pad = np.full(NREL, rel_ids[0], dtype=np.int64)
    for s, rid in enumerate(rel_ids):
        masks[s] = (et == rid)
        rel_pad[s] = rid
    # padded slots keep zero masks -> contribute nothing

    W_used = np.ascontiguousarray(np.asarray(W_rel)[rel_pad], dtype=f32)  # [8,256,256]

    xt = np.ascontiguousarray(x.T).reshape(2, 128, L)
    wq = np.ascontiguousarray(Wq, dtype=f32).reshape(2, 128, A)
    wk = np.ascontiguousarray(Wk, dtype=f32).reshape(2, 128, A)
    maskw = np.ascontiguousarray(masks.transpose(1, 0, 2)).reshape(L, NREL * L)
    apack = np.ascontiguousarray(np.concatenate([xt, wq, wk], axis=2))
    W_root = np.asarray(W_root, dtype=f32)
    W_self = np.asarray(W_self, dtype=f32)
    W_nbr = np.asarray(W_nbr, dtype=f32)
    b_rgcn = np.asarray(b_rgcn, dtype=f32)
    b_gcn = np.asarray(b_gcn, dtype=f32)

    in_maps = []
    for c in range(N_CORES):
        sl = slice(c * HS, (c + 1) * HS)
        wrel_c = np.ascontiguousarray(
            W_used[:, :, sl].transpose(1, 0, 2)).reshape(2, 128, NREL * HS)
        rpack = np.ascontiguousarray(np.concatenate([
            wrel_c,
            W_root[:, sl].reshape(2, 128, HS),
            W_self[:, sl].reshape(2, 128, HS),
            W_nbr[:, sl].reshape(2, 128, HS),
        ], axis=2))
        spack = np.zeros((128, 3 + NREL * L), dtype=f32)
        spack[:, 0] = np.ascontiguousarray(v, dtype=f32).reshape(128)
        spack[0:L, 1:1 + NREL * L] = maskw
        spack[0:HS, 1 + NREL * L] = b_rgcn[sl]
        spack[0:HS, 2 + NREL * L] = b_gcn[sl]
        in_maps.append({"apack": apack, "rpack": rpack, "spack": spack})
    return in_maps


def kernel(global_features, speaker, Wq, Wk, v, W_rel, W_root, b_rgcn,
           W_nbr, W_self, b_gcn):
    global _compiled
    from concourse.bass_utils import run_bass_kernel_spmd

    if _compiled is None:
        _compiled = build_program()
    nc = _compiled

    in_maps = _prepare_in_maps(global_features, speaker, Wq, Wk, v, W_rel,
                               W_root, b_rgcn, W_nbr, W_self, b_gcn)
    res = run_bass_kernel_spmd(nc, in_maps, core_ids=list(range(N_CORES)))
    outT = np.concatenate([res.results[c]["yout"] for c in range(N_CORES)], axis=0)
    return np.ascontiguousarray(outT.T)


# revision 26
# speedup vs baseline: 1.2332x; 1.2332x over previous
"""DialogueGCN forward pass as a distributed Bass/Tile kernel on 8 TRN2 NeuronCores.

Math (reference): Bahdanau attention over utterance pairs -> per-edge softmax
weights; RGCN with per-relation weights W_rel[etype] + root term; GraphConv.

Key structural facts exploited:
  * etype = 2*(sp[i]*48 + sp[j]) + (i>=j) with speaker in {0,1} -> at most 8
    distinct relation types out of R=4608 are live. Only those 8 slices of the
    1.2GB W_rel are ever touched (host routes them to the devices).
  * The graph is fully connected, so the GraphConv neighbor sum is the same
    vector for every node: sum_i h_i.
  * agg = sum_r (attn*mask_r)^T (x @ W_r): 8 masked matmuls instead of a
    2304-edge gather/scatter.

Sharding: the RGCN/GraphConv hidden dim H=256 is split 8 ways (32 columns per
core); every core computes the full [48,48] attention (cheap, avoids a second
collective), its h-slice of the RGCN, then one AllGather of the [32,48] slices
rebuilds h^T [256,48] on every core, and each core finishes its g-slice of the
GraphConv output. Host concatenates the 8 [32,48] outputs and transposes.
"""
import numpy as np

L = 48
D = 256
H = 256
G = 256
A = 128
N_CORES = 8
HS = H // N_CORES  # 32 columns of h (and of the output) per core
NREL = 8

_compiled = None


def _emit_body(nc, mybir, pool, psum, dram, d, rep, collective, n_cores):
    """Emit one full forward pass. `d` maps dram-tensor names to handles."""
    dt = mybir.dt.float32
    u = f"_{rep}"

    # ---- three packed input DMAs, on three different engine queues ----
    apk = pool.tile([128, 2, 304], mybir.dt.float32r, name=f"apk{u}", tag="apk")
    rpk = pool.tile([128, 2, 352], mybir.dt.float32r, name=f"rpk{u}", tag="rpk")
    spk = pool.tile([128, 387], dt, name=f"spk{u}", tag="spk")
    for t in range(2):
        nc.sync.dma_start(apk[:, t, :], d["apack"].ap()[t])
        nc.gpsimd.dma_start(rpk[:, t, :], d["rpack"].ap()[t])
    nc.scalar.dma_start(spk[:], d["spack"].ap())

    def xt(t):
        return apk[:, t, 0:L]
    def wqs(t):
        return apk[:, t, L:L + A]
    def wks(t):
        return apk[:, t, L + A:L + 2 * A]
    def wr(t):
        return rpk[:, t, 0:NREL * HS]
    def wro(t):
        return rpk[:, t, NREL * HS:NREL * HS + HS]
    def wsl(t):
        return rpk[:, t, NREL * HS + HS:NREL * HS + 2 * HS]
    def wn(t):
        return rpk[:, t, NREL * HS + 2 * HS:NREL * HS + 3 * HS]
    vv = spk[:, 0:1]
    maskt = spk[0:L, 1:1 + NREL * L].rearrange("i (r j) -> i r j", r=NREL)
    brg = spk[0:HS, 1 + NREL * L:2 + NREL * L]
    bgc = spk[0:HS, 2 + NREL * L:3 + NREL * L]

    # f32r (TF32-like) matmul operands: 4x PE rate on wide outputs, ~1e-3
    # matmul precision -- well inside the output tolerance here.
    f32r = mybir.dt.float32r
    # ---- Bahdanau attention: scores[i,j] = v . tanh(qT[:,i]+kT[:,j]) ----
    # qT/kT stay in PSUM; the broadcast add reads them there directly
    qT_ps = psum.tile([128, L, 1], dt, name=f"qT_ps{u}", tag="qt_ps")
    for t in range(2):
        nc.tensor.matmul(qT_ps[:, :, 0], wqs(t), xt(t), start=(t == 0), stop=(t == 1))
    kT_ps = psum.tile([128, 1, L], dt, name=f"kT_ps{u}", tag="kt_ps")
    for t in range(2):
        nc.tensor.matmul(kT_ps[:, 0, :], wks(t), xt(t), start=(t == 0), stop=(t == 1))
    kTs = pool.tile([128, 1, L], dt, name=f"kTs{u}", tag="kTs")
    nc.vector.tensor_copy(kTs[:, 0, :], kT_ps[:, 0, :])

    # broadcast add + tanh + v-matmul, in 2 halves so DVE/ACT/PE pipeline
    HL = L // 2  # 24 query rows per half
    bigT = pool.tile([128, L, L], dt, name=f"bigT{u}", tag="bigT")
    # tanh output + v are written as float32r so the score matmuls can run
    # the 4x-rate f32r PE path (producers must round to f32r per verifier)
    tanhT = pool.tile([128, L * L], f32r, name=f"tanhT{u}", tag="tanhT")
    vvr = pool.tile([128, 1], f32r, name=f"vvr{u}", tag="vvr")
    nc.vector.tensor_copy(vvr[:], vv)
    scores_ps = psum.tile([1, L * L], dt, name=f"scores_ps{u}", tag="attn_big")
    for h in range(2):
        isl = slice(h * HL, (h + 1) * HL)
        csl = slice(h * HL * L, (h + 1) * HL * L)
        nc.vector.tensor_tensor(
            bigT[:, isl, :],
            qT_ps[:, isl, :].broadcast_to([128, HL, L]),
            kTs[:].broadcast_to([128, HL, L]),
            op=mybir.AluOpType.add,
        )
        nc.scalar.activation(tanhT[:, csl],
                             bigT[:, isl, :].rearrange("p i j -> p (i j)"),
                             mybir.ActivationFunctionType.Tanh)
        # matmul outputs must stay inside one PSUM bank: chunk on global
        # 512-aligned boundaries; emit the chunks fully covered by the tanh
        # ready so far (half 0: [0:1024]; half 1: the rest).
        lo = 1024 if h == 1 else 0
        hi_lim = HL * L * (h + 1) if h == 0 else L * L
        for k in range(lo, hi_lim - 511 if h == 0 else hi_lim, 512):
            hi = min(k + 512, L * L)
            nc.tensor.matmul(scores_ps[:, k:hi], vvr[:], tanhT[:, k:hi],
                             start=True, stop=True)
    # PSUM -> SBUF [1, 2304]: single-partition copy, per-chunk, alternating
    # DVE/ACT so it pipelines behind the score matmuls
    scores_row = pool.tile([1, L * L], dt, name=f"scores_row{u}", tag="scores_row")
    for ci, k in enumerate(range(0, L * L, 512)):
        hi = min(k + 512, L * L)
        if ci % 2 == 0:
            nc.vector.tensor_copy(scores_row[:, k:hi], scores_ps[:, k:hi])
        else:
            nc.scalar.copy(scores_row[:, k:hi], scores_ps[:, k:hi])
    # reshape [1, 2304] -> [48, 48] has to cross partitions: DRAM bounce
    scores_dram = dram.tile([1, L * L], dt, name=f"scores_dram{u}", tag="scores_dram")
    nc.sync.dma_start(scores_dram[:], scores_row[:])
    scores = pool.tile([L, L], dt, name=f"scores{u}", tag="scores")
    sd_half = scores_dram[:].rearrange("p (a i j) -> a (p i) j", a=2, i=L // 2)
    nc.sync.dma_start(scores[0:L // 2, :], sd_half[0])
    nc.scalar.dma_start(scores[L // 2:L, :], sd_half[1])

    # ---- softmax over j (scores bounded by sum|v| ~ 9: no max pass) ----
    expS = pool.tile([L, L], dt, name=f"expS{u}", tag="expS")
    rowsum = pool.tile([L, 1], dt, name=f"rowsum{u}", tag="rowsum")
    nc.scalar.activation(expS[:], scores[:], mybir.ActivationFunctionType.Exp,
                         accum_out=rowsum[:])
    recip = pool.tile([L, 1], dt, name=f"recip{u}", tag="recip")
    nc.vector.reciprocal(recip[:], rowsum[:])
    # A_r[i,j] = (exp * 1/rowsum) * mask_r, all 8 relations in one op
    attnW = pool.tile([L, NREL, L], dt, name=f"attnW{u}", tag="attnW")
    NH = NREL // 2
    for a in range(2):
        rsl = slice(a * NH, (a + 1) * NH)
        nc.vector.scalar_tensor_tensor(
            attnW[:, rsl, :],
            expS[:].rearrange("i (o j) -> i o j", o=1).broadcast_to([L, NH, L]),
            recip[:],
            maskt[:, rsl, :],
            op0=mybir.AluOpType.mult,
            op1=mybir.AluOpType.mult,
        )

    # ---- RGCN (h-slice): Yall[i, r*32+h'] = (x @ W_r[:, hsl]) ----
    yall_ps = psum.tile([L, NREL * HS], dt, name=f"yall_ps{u}", tag="mm_ps")
    for t in range(2):
        nc.tensor.matmul(yall_ps[:], xt(t), wr(t), start=(t == 0), stop=(t == 1))
    yall = pool.tile([L, NREL * HS], dt, name=f"yall{u}", tag="yall")
    nc.vector.tensor_copy(yall[:], yall_ps[:])

    # hT_slice[h', j] = sum_r sum_i Y_r[i,h'] A_r[i,j] + (x@W_root)^T + b
    h_ps = psum.tile([HS, L], dt, name=f"h_ps{u}", tag="mm_ps")
    for r in range(NREL):
        nc.tensor.matmul(h_ps[:], yall[:, r * HS:(r + 1) * HS], attnW[:, r, :],
                         start=(r == 0), stop=False)
    for t in range(2):
        nc.tensor.matmul(h_ps[:], wro(t).bitcast(dt), xt(t).bitcast(dt),
                         start=False, stop=(t == 1))
    hTs = pool.tile([HS, L], dt, name=f"hTs{u}", tag="hTs")
    nc.vector.tensor_scalar_add(hTs[:], h_ps[:], brg)

    # ---- AllGather h-slices -> full hT [256, 48] on every core ----
    ag_in = dram.tile([HS, L], dt, name=f"ag_in{u}", tag="ag_in")
    ag_out = dram.tile([H, L], dt, name=f"ag_out{u}", tag="ag_out")
    nc.sync.dma_start(ag_in[:], hTs[:])
    if collective:
        nc.gpsimd.collective_compute(
            "AllGather",
            mybir.AluOpType.bypass,
            replica_groups=[list(range(n_cores))],
            ins=[ag_in.opt()],
            outs=[ag_out.opt()],
        )
    else:
        # single-core stand-in for TimelineSim: replicate the slice 8x
        agw = ag_out[:].rearrange("(c p) f -> c p f", p=HS)
        for c in range(N_CORES):
            nc.sync.dma_start(agw[c], ag_in[:])
    hfull = pool.tile([128, 2, L], dt, name=f"hfull{u}", tag="hfull")
    agv = ag_out[:].rearrange("(t p) f -> t p f", p=128)
    nc.sync.dma_start(hfull[:, 0, :], agv[0])
    nc.scalar.dma_start(hfull[:, 1, :], agv[1])

    # ---- GraphConv (g-slice): out^T = W_self^T hT + (W_nbr^T s + b) ----
    sT = pool.tile([128, 2], dt, name=f"sT{u}", tag="sT")
    for t in range(2):
        nc.vector.reduce_sum(sT[:, t:t + 1], hfull[:, t, :],
                             axis=mybir.AxisListType.X)
    nb_ps = psum.tile([HS, 1], dt, name=f"nb_ps{u}", tag="mm_ps")
    for t in range(2):
        nc.tensor.matmul(nb_ps[:], wn(t).bitcast(dt), sT[:, t:t + 1],
                         start=(t == 0), stop=(t == 1))
    nbs = pool.tile([HS, 1], dt, name=f"nbs{u}", tag="nbs")
    nc.vector.tensor_scalar_add(nbs[:], nb_ps[:], bgc)

    out_ps = psum.tile([HS, L], dt, name=f"out_ps{u}", tag="mm_ps")
    for t in range(2):
        nc.tensor.matmul(out_ps[:], wsl(t).bitcast(dt), hfull[:, t, :],
                         start=(t == 0), stop=(t == 1))
    outs = pool.tile([HS, L], dt, name=f"outs{u}", tag="outs")
    nc.vector.tensor_scalar_add(outs[:], out_ps[:], nbs)
    nc.sync.dma_start(d["yout"].ap(), outs[:])


def build_program(n_cores=N_CORES, collective=True, repeat=1):
    """Build + schedule + compile the Bass program."""
    import concourse.bacc as bacc
    import concourse.mybir as mybir
    import concourse.tile as tile

    dt = mybir.dt.float32
    nc = bacc.Bacc("TRN2", debug=False, num_devices=n_cores)

    d = {}
    d["apack"] = nc.dram_tensor("apack", [2, 128, 304], mybir.dt.float32r,
                            kind="ExternalInput")
    d["rpack"] = nc.dram_tensor("rpack", [2, 128, 352], mybir.dt.float32r,
                            kind="ExternalInput")
    d["spack"] = nc.dram_tensor("spack", [128, 387], dt, kind="ExternalInput")
    d["yout"] = nc.dram_tensor("yout", [HS, L], dt, kind="ExternalOutput")

    with tile.TileContext(nc) as tc:
        with (
            tc.tile_pool(name="sbuf", bufs=1) as pool,
            tc.tile_pool(name="psum", bufs=1, space="PSUM") as psum,
            tc.tile_pool(name="dram", bufs=1, space="DRAM") as dram,
        ):
            for rep in range(repeat):
                _emit_body(nc, mybir, pool, psum, dram, d, rep, collective, n_cores)

    nc.compile()
    return nc


def _prepare_in_maps(global_features, speaker, Wq, Wk, v, W_rel, W_root, b_rgcn,
                     W_nbr, W_self, b_gcn):
    """Host-side routing: pick the <=8 live relation slices, build masks, pack
    per-core shards (h-slice of RGCN weights, g-slice of GraphConv weights)."""
    f32 = np.float32
    x = np.ascontiguousarray(global_features, dtype=f32)
    sp = np.asarray(speaker).astype(np.int64)
    n = L

    ii, jj = np.meshgrid(np.arange(n), np.arange(n), indexing="ij")
    direction = (ii >= jj).astype(np.int64)
    et = 2 * (sp[ii] * n + sp[jj]) + direction  # [48, 48] edge-type grid

    rel_ids = np.unique(et)
    assert len(rel_ids) <= NREL, f"{len(rel_ids)} live relations > {NREL}"
    masks = np.zeros((NREL, n, n), dtype=f32)
    rel_pad = np.full(NREL, rel_ids[0], dtype=np.int64)
    for s, rid in enumerate(rel_ids):
        masks[s] = (et == rid)
        rel_pad[s] = rid
    # padded slots keep zero masks -> contribute nothing

    W_used = np.ascontiguousarray(np.asarray(W_rel)[rel_pad], dtype=f32)  # [8,256,256]

    xt = np.ascontiguousarray(x.T).reshape(2, 128, L)
    wq = np.ascontiguousarray(Wq, dtype=f32).reshape(2, 128, A)
    wk = np.ascontiguousarray(Wk, dtype=f32).reshape(2, 128, A)
    maskw = np.ascontiguousarray(masks.transpose(1, 0, 2)).reshape(L, NREL * L)
    apack = np.ascontiguousarray(np.concatenate([xt, wq, wk], axis=2))
    W_root = np.asarray(W_root, dtype=f32)
    W_self = np.asarray(W_self, dtype=f32)
    W_nbr = np.asarray(W_nbr, dtype=f32)
    b_rgcn = np.asarray(b_rgcn, dtype=f32)
    b_gcn = np.asarray(b_gcn, dtype=f32)

    in_maps = []
    for c in range(N_CORES):
        sl = slice(c * HS, (c + 1) * HS)
        wrel_c = np.ascontiguousarray(
            W_used[:, :, sl].transpose(1, 0, 2)).reshape(2, 128, NREL * HS)
        rpack = np.ascontiguousarray(np.concatenate([
            wrel_c,
            W_root[:, sl].reshape(2, 128, HS),
            W_self[:, sl].reshape(2, 128, HS),
            W_nbr[:, sl].reshape(2, 128, HS),
        ], axis=2))
        spack = np.zeros((128, 3 + NREL * L), dtype=f32)
        spack[:, 0] = np.ascontiguousarray(v, dtype=f32).reshape(128)
        spack[0:L, 1:1 + NREL * L] = maskw
        spack[0:HS, 1 + NREL * L] = b_rgcn[sl]
        spack[0:HS, 2 + NREL * L] = b_gcn[sl]
        in_maps.append({"apack": apack, "rpack": rpack, "spack": spack})
    return in_maps


def kernel(global_features, speaker, Wq, Wk, v, W_rel, W_root, b_rgcn,
           W_nbr, W_self, b_gcn):
    global _compiled
    from concourse.bass_utils import run_bass_kernel_spmd

    if _compiled is None:
        _compiled = build_program()
    nc = _compiled

    in_maps = _prepare_in_maps(global_features, speaker, Wq, Wk, v, W_rel,
                               W_root, b_rgcn, W_nbr, W_self, b_gcn)
    res = run_bass_kernel_spmd(nc, in_maps, core_ids=list(range(N_CORES)))
    outT = np.concatenate([res.results[c]["yout"] for c in range(N_CORES)], axis=0)
    return np.ascontiguousarray(outT.T)


# revision 28
# speedup vs baseline: 1.4018x; 1.1368x over previous
"""DialogueGCN forward pass as a distributed Bass/Tile kernel on 8 TRN2 NeuronCores.

Math (reference): Bahdanau attention over utterance pairs -> per-edge softmax
weights; RGCN with per-relation weights W_rel[etype] + root term; GraphConv.

Key structural facts exploited:
  * etype = 2*(sp[i]*48 + sp[j]) + (i>=j) with speaker in {0,1} -> at most 8
    distinct relation types out of R=4608 are live. Only those 8 slices of the
    1.2GB W_rel are ever touched (host routes them to the devices).
  * The graph is fully connected, so the GraphConv neighbor sum is the same
    vector for every node: sum_i h_i.
  * agg = sum_r (attn*mask_r)^T (x @ W_r): 8 masked matmuls instead of a
    2304-edge gather/scatter.

Sharding: the RGCN/GraphConv hidden dim H=256 is split 8 ways (32 columns per
core); every core computes the full [48,48] attention (cheap, avoids a second
collective), its h-slice of the RGCN, then one AllGather of the [32,48] slices
rebuilds h^T [256,48] on every core, and each core finishes its g-slice of the
GraphConv output. Host concatenates the 8 [32,48] outputs and transposes.
"""
import numpy as np

L = 48
D = 256
H = 256
G = 256
A = 128
N_CORES = 8
HS = H // N_CORES  # 32 columns of h (and of the output) per core
NREL = 8

_compiled = None


def _emit_body(nc, mybir, pool, psum, dram, d, rep, collective, n_cores):
    """Emit one full forward pass. `d` maps dram-tensor names to handles."""
    dt = mybir.dt.float32
    u = f"_{rep}"

    # ---- three packed input DMAs, on three different engine queues ----
    apk = pool.tile([128, 2, 304], mybir.dt.float32r, name=f"apk{u}", tag="apk")
    rpk = pool.tile([128, 2, 352], mybir.dt.float32r, name=f"rpk{u}", tag="rpk")
    spk = pool.tile([128, 387], dt, name=f"spk{u}", tag="spk")
    for t in range(2):
        nc.sync.dma_start(apk[:, t, :], d["apack"].ap()[t])
        nc.gpsimd.dma_start(rpk[:, t, :], d["rpack"].ap()[t])
    nc.scalar.dma_start(spk[:], d["spack"].ap())

    def xt(t):
        return apk[:, t, 0:L]
    def wqs(t):
        return apk[:, t, L:L + A]
    def wks(t):
        return apk[:, t, L + A:L + 2 * A]
    def wr(t):
        return rpk[:, t, 0:NREL * HS]
    def wro(t):
        return rpk[:, t, NREL * HS:NREL * HS + HS]
    def wsl(t):
        return rpk[:, t, NREL * HS + HS:NREL * HS + 2 * HS]
    def wn(t):
        return rpk[:, t, NREL * HS + 2 * HS:NREL * HS + 3 * HS]
    vv = spk[:, 0:1]
    maskt = spk[0:L, 1:1 + NREL * L].rearrange("i (r j) -> i r j", r=NREL)
    brg = spk[0:HS, 1 + NREL * L:2 + NREL * L]
    bgc = spk[0:HS, 2 + NREL * L:3 + NREL * L]

    # f32r (TF32-like) matmul operands: 4x PE rate on wide outputs, ~1e-3
    # matmul precision -- well inside the output tolerance here.
    f32r = mybir.dt.float32r
    # ---- Bahdanau attention: scores[i,j] = v . tanh(qT[:,i]+kT[:,j]) ----
    # qT/kT stay in PSUM; the broadcast add reads them there directly
    qT_ps = psum.tile([128, L, 1], dt, name=f"qT_ps{u}", tag="qt_ps")
    for t in range(2):
        nc.tensor.matmul(qT_ps[:, :, 0], wqs(t), xt(t), start=(t == 0), stop=(t == 1))
    kT_ps = psum.tile([128, 1, L], dt, name=f"kT_ps{u}", tag="kt_ps")
    for t in range(2):
        nc.tensor.matmul(kT_ps[:, 0, :], wks(t), xt(t), start=(t == 0), stop=(t == 1))
    kTs = pool.tile([128, 1, L], dt, name=f"kTs{u}", tag="kTs")
    nc.vector.tensor_copy(kTs[:, 0, :], kT_ps[:, 0, :])

    # broadcast add + tanh + v-matmul, in 2 halves so DVE/ACT/PE pipeline
    HL = L // 2  # 24 query rows per half
    bigT = pool.tile([128, L, L], dt, name=f"bigT{u}", tag="bigT")
    # tanh output + v are written as float32r so the score matmuls can run
    # the 4x-rate f32r PE path (producers must round to f32r per verifier)
    tanhT = pool.tile([128, L * L], f32r, name=f"tanhT{u}", tag="tanhT")
    vvr = pool.tile([128, 1], f32r, name=f"vvr{u}", tag="vvr")
    nc.vector.tensor_copy(vvr[:], vv)
    scores_ps = psum.tile([1, L * L], dt, name=f"scores_ps{u}", tag="attn_big")
    for h in range(2):
        isl = slice(h * HL, (h + 1) * HL)
        csl = slice(h * HL * L, (h + 1) * HL * L)
        nc.vector.tensor_tensor(
            bigT[:, isl, :],
            qT_ps[:, isl, :].broadcast_to([128, HL, L]),
            kTs[:].broadcast_to([128, HL, L]),
            op=mybir.AluOpType.add,
        )
        nc.scalar.activation(tanhT[:, csl],
                             bigT[:, isl, :].rearrange("p i j -> p (i j)"),
                             mybir.ActivationFunctionType.Tanh)
        # matmul outputs must stay inside one PSUM bank: chunk on global
        # 512-aligned boundaries; emit the chunks fully covered by the tanh
        # ready so far (half 0: [0:1024]; half 1: the rest).
        lo = 1024 if h == 1 else 0
        hi_lim = HL * L * (h + 1) if h == 0 else L * L
        for k in range(lo, hi_lim - 511 if h == 0 else hi_lim, 512):
            hi = min(k + 512, L * L)
            nc.tensor.matmul(scores_ps[:, k:hi], vvr[:], tanhT[:, k:hi],
                             start=True, stop=True)
    # PSUM -> SBUF [1, 2304]: single-partition copy, per-chunk, alternating
    # DVE/ACT so it pipelines behind the score matmuls
    scores_row = pool.tile([1, L * L], dt, name=f"scores_row{u}", tag="scores_row")
    for ci, k in enumerate(range(0, L * L, 512)):
        hi = min(k + 512, L * L)
        if ci % 2 == 0:
            nc.vector.tensor_copy(scores_row[:, k:hi], scores_ps[:, k:hi])
        else:
            nc.scalar.copy(scores_row[:, k:hi], scores_ps[:, k:hi])
    # reshape [1, 2304] -> [48, 48] has to cross partitions: DRAM bounce
    scores_dram = dram.tile([1, L * L], dt, name=f"scores_dram{u}", tag="scores_dram")
    nc.sync.dma_start(scores_dram[:], scores_row[:])
    scores = pool.tile([L, L], dt, name=f"scores{u}", tag="scores")
    sd_half = scores_dram[:].rearrange("p (a i j) -> a (p i) j", a=2, i=L // 2)
    nc.sync.dma_start(scores[0:L // 2, :], sd_half[0])
    nc.scalar.dma_start(scores[L // 2:L, :], sd_half[1])

    # ---- softmax over j (scores bounded by sum|v| ~ 9: no max pass) ----
    expS = pool.tile([L, L], dt, name=f"expS{u}", tag="expS")
    rowsum = pool.tile([L, 1], dt, name=f"rowsum{u}", tag="rowsum")
    nc.scalar.activation(expS[:], scores[:], mybir.ActivationFunctionType.Exp,
                         accum_out=rowsum[:])
    recip = pool.tile([L, 1], dt, name=f"recip{u}", tag="recip")
    nc.vector.reciprocal(recip[:], rowsum[:])
    # A_r[i,j] = (exp * 1/rowsum) * mask_r, all 8 relations in one op
    attnW = pool.tile([L, NREL, L], dt, name=f"attnW{u}", tag="attnW")
    NH = NREL // 2
    for a in range(2):
        rsl = slice(a * NH, (a + 1) * NH)
        nc.vector.scalar_tensor_tensor(
            attnW[:, rsl, :],
            expS[:].rearrange("i (o j) -> i o j", o=1).broadcast_to([L, NH, L]),
            recip[:],
            maskt[:, rsl, :],
            op0=mybir.AluOpType.mult,
            op1=mybir.AluOpType.mult,
        )

    # ---- RGCN (h-slice): Yall[i, r*32+h'] = (x @ W_r[:, hsl]) ----
    yall_ps = psum.tile([L, NREL * HS], dt, name=f"yall_ps{u}", tag="mm_ps")
    for t in range(2):
        nc.tensor.matmul(yall_ps[:], xt(t), wr(t), start=(t == 0), stop=(t == 1))
    yall = pool.tile([L, NREL * HS], dt, name=f"yall{u}", tag="yall")
    nc.vector.tensor_copy(yall[:], yall_ps[:])

    # hT_slice[h', j] = sum_r sum_i Y_r[i,h'] A_r[i,j] + (x@W_root)^T + b
    h_ps = psum.tile([HS, L], dt, name=f"h_ps{u}", tag="mm_ps")
    for r in range(NREL):
        nc.tensor.matmul(h_ps[:], yall[:, r * HS:(r + 1) * HS], attnW[:, r, :],
                         start=(r == 0), stop=False)
    for t in range(2):
        nc.tensor.matmul(h_ps[:], wro(t).bitcast(dt), xt(t).bitcast(dt),
                         start=False, stop=(t == 1))
    hTs = pool.tile([HS, L], dt, name=f"hTs{u}", tag="hTs")
    nc.vector.tensor_scalar_add(hTs[:], h_ps[:], brg)

    # ---- AllGather h-slices -> full hT [256, 48] on every core ----
    ag_in = dram.tile([HS, L], dt, name=f"ag_in{u}", tag="ag_in")
    ag_out = dram.tile([H, L], dt, name=f"ag_out{u}", tag="ag_out")
    nc.sync.dma_start(ag_in[:], hTs[:])
    if collective:
        nc.gpsimd.collective_compute(
            "AllGather",
            mybir.AluOpType.bypass,
            replica_groups=[list(range(n_cores))],
            ins=[ag_in.opt()],
            outs=[ag_out.opt()],
        )
    else:
        # single-core stand-in for TimelineSim: replicate the slice 8x
        agw = ag_out[:].rearrange("(c p) f -> c p f", p=HS)
        for c in range(N_CORES):
            nc.sync.dma_start(agw[c], ag_in[:])
    hfull = pool.tile([128, 2, L], dt, name=f"hfull{u}", tag="hfull")
    agv = ag_out[:].rearrange("(t p) f -> t p f", p=128)
    nc.sync.dma_start(hfull[:, 0, :], agv[0])
    nc.scalar.dma_start(hfull[:, 1, :], agv[1])

    # ---- GraphConv (g-slice): out^T = W_self^T hT + (W_nbr^T s + b) ----
    sT = pool.tile([128, 2], dt, name=f"sT{u}", tag="sT")
    for t in range(2):
        nc.vector.reduce_sum(sT[:, t:t + 1], hfull[:, t, :],
                             axis=mybir.AxisListType.X)
    nb_ps = psum.tile([HS, 1], dt, name=f"nb_ps{u}", tag="mm_ps")
    for t in range(2):
        nc.tensor.matmul(nb_ps[:], wn(t).bitcast(dt), sT[:, t:t + 1],
                         start=(t == 0), stop=(t == 1))
    nbs = pool.tile([HS, 1], dt, name=f"nbs{u}", tag="nbs")
    nc.vector.tensor_scalar_add(nbs[:], nb_ps[:], bgc)

    out_ps = psum.tile([HS, L], dt, name=f"out_ps{u}", tag="mm_ps")
    for t in range(2):
        nc.tensor.matmul(out_ps[:], wsl(t).bitcast(dt), hfull[:, t, :],
                         start=(t == 0), stop=(t == 1))
    outs = pool.tile([HS, L], dt, name=f"outs{u}", tag="outs")
    nc.vector.tensor_scalar_add(outs[:], out_ps[:], nbs)
    nc.sync.dma_start(d["yout"].ap(), outs[:])


def build_program(n_cores=N_CORES, collective=True, repeat=1):
    """Build + schedule + compile the Bass program."""
    import concourse.bacc as bacc
    import concourse.mybir as mybir
    import concourse.tile as tile

    dt = mybir.dt.float32
    nc = bacc.Bacc("TRN2", debug=False, num_devices=n_cores)

    d = {}
    d["apack"] = nc.dram_tensor("apack", [2, 128, 304], mybir.dt.float32r,
                            kind="ExternalInput")
    d["rpack"] = nc.dram_tensor("rpack", [2, 128, 352], mybir.dt.float32r,
                            kind="ExternalInput")
    d["spack"] = nc.dram_tensor("spack", [128, 387], dt, kind="ExternalInput")
    d["yout"] = nc.dram_tensor("yout", [HS, L], dt, kind="ExternalOutput")

    with tile.TileContext(nc) as tc:
        with (
            tc.tile_pool(name="sbuf", bufs=1) as pool,
            tc.tile_pool(name="psum", bufs=1, space="PSUM") as psum,
            tc.tile_pool(name="dram", bufs=1, space="DRAM") as dram,
        ):
            for rep in range(repeat):
                _emit_body(nc, mybir, pool, psum, dram, d, rep, collective, n_cores)

    nc.compile()
    return nc


def _prepare_in_maps(global_features, speaker, Wq, Wk, v, W_rel, W_root, b_rgcn,
                     W_nbr, W_self, b_gcn):
    """Host-side routing: pick the <=8 live relation slices, build masks, pack
    per-core shards (h-slice of RGCN weights, g-slice of GraphConv weights)."""
    f32 = np.float32
    x = np.ascontiguousarray(global_features, dtype=f32)
    sp = np.asarray(speaker).astype(np.int64)
    n = L

    ii, jj = np.meshgrid(np.arange(n), np.arange(n), indexing="ij")
    direction = (ii >= jj).astype(np.int64)
    et = 2 * (sp[ii] * n + sp[jj]) + direction  # [48, 48] edge-type grid

    rel_ids = np.unique(et)
    assert len(rel_ids) <= NREL, f"{len(rel_ids)} live relations > {NREL}"
    masks = np.zeros((NREL, n, n), dtype=f32)
    rel_pad = np.full(NREL, rel_ids[0], dtype=np.int64)
    for s, rid in enumerate(rel_ids):
        masks[s] = (et == rid)
        rel_pad[s] = rid
    # padded slots keep zero masks -> contribute nothing

    W_used = np.ascontiguousarray(np.asarray(W_rel)[rel_pad], dtype=f32)  # [8,256,256]

    xt = np.ascontiguousarray(x.T).reshape(2, 128, L)
    wq = np.ascontiguousarray(Wq, dtype=f32).reshape(2, 128, A)
    wk = np.ascontiguousarray(Wk, dtype=f32).reshape(2, 128, A)
    maskw = np.ascontiguousarray(masks.transpose(1, 0, 2)).reshape(L, NREL * L)
    apack = np.ascontiguousarray(np.concatenate([xt, wq, wk], axis=2))
    W_root = np.asarray(W_root, dtype=f32)
    W_self = np.asarray(W_self, dtype=f32)
    W_nbr = np.asarray(W_nbr, dtype=f32)
    b_rgcn = np.asarray(b_rgcn, dtype=f32)
    b_gcn = np.asarray(b_gcn, dtype=f32)

    in_maps = []
    for c in range(N_CORES):
        sl = slice(c * HS, (c + 1) * HS)
        wrel_c = np.ascontiguousarray(
            W_used[:, :, sl].transpose(1, 0, 2)).reshape(2, 128, NREL * HS)
        rpack = np.ascontiguousarray(np.concatenate([
            wrel_c,
            W_root[:, sl].reshape(2, 128, HS),
            W_self[:, sl].reshape(2, 128, HS),
            W_nbr[:, sl].reshape(2, 128, HS),
        ], axis=2))
        spack = np.zeros((128, 3 + NREL * L), dtype=f32)
        spack[:, 0] = np.ascontiguousarray(v, dtype=f32).reshape(128)
        spack[0:L, 1:1 + NREL * L] = maskw
        spack[0:HS, 1 + NREL * L] = b_rgcn[sl]
        spack[0:HS, 2 + NREL * L] = b_gcn[sl]
        in_maps.append({"apack": apack, "rpack": rpack, "spack": spack})
    return in_maps


def kernel(global_features, speaker, Wq, Wk, v, W_rel, W_root, b_rgcn,
           W_nbr, W_self, b_gcn):
    global _compiled
    from concourse.bass_utils import run_bass_kernel_spmd

    if _compiled is None:
        _compiled = build_program()
    nc = _compiled

    in_maps = _prepare_in_maps(global_features, speaker, Wq, Wk, v, W_rel,
                               W_root, b_rgcn, W_nbr, W_self, b_gcn)
    res = run_bass_kernel_spmd(nc, in_maps, core_ids=list(range(N_CORES)))
    outT = np.concatenate([res.results[c]["yout"] for c in range(N_CORES)], axis=0)
    return np.ascontiguousarray(outT.T)


# revision 29
# speedup vs baseline: 1.8763x; 1.3385x over previous
"""DialogueGCN forward pass as a distributed Bass/Tile kernel on 8 TRN2 NeuronCores.

Math (reference): Bahdanau attention over utterance pairs -> per-edge softmax
weights; RGCN with per-relation weights W_rel[etype] + root term; GraphConv.

Key structural facts exploited:
  * etype = 2*(sp[i]*48 + sp[j]) + (i>=j) with speaker in {0,1} -> at most 8
    distinct relation types out of R=4608 are live. Only those 8 slices of the
    1.2GB W_rel are ever touched (host routes them to the devices).
  * The graph is fully connected, so the GraphConv neighbor sum is the same
    vector for every node: sum_i h_i.
  * agg = sum_r (attn*mask_r)^T (x @ W_r): 8 masked matmuls instead of a
    2304-edge gather/scatter.

Sharding: the RGCN/GraphConv hidden dim H=256 is split 8 ways (32 columns per
core); every core computes the full [48,48] attention (cheap, avoids a second
collective), its h-slice of the RGCN, then one AllGather of the [32,48] slices
rebuilds h^T [256,48] on every core, and each core finishes its g-slice of the
GraphConv output. Host concatenates the 8 [32,48] outputs and transposes.
"""
import numpy as np

L = 48
D = 256
H = 256
G = 256
A = 128
N_CORES = 8
HS = H // N_CORES  # 32 columns of h (and of the output) per core
NREL = 8

_compiled = None


def _emit_body(nc, mybir, pool, psum, dram, d, rep, collective, n_cores):
    """Emit one full forward pass. `d` maps dram-tensor names to handles."""
    dt = mybir.dt.float32
    u = f"_{rep}"

    # ---- three packed input DMAs, on three different engine queues ----
    apk = pool.tile([128, 2, 304], mybir.dt.float32r, name=f"apk{u}", tag="apk")
    rpk = pool.tile([128, 2, 352], mybir.dt.float32r, name=f"rpk{u}", tag="rpk")
    spk = pool.tile([128, 387], dt, name=f"spk{u}", tag="spk")
    for t in range(2):
        nc.sync.dma_start(apk[:, t, :], d["apack"].ap()[t])
        nc.gpsimd.dma_start(rpk[:, t, :], d["rpack"].ap()[t])
    nc.scalar.dma_start(spk[:], d["spack"].ap())

    def xt(t):
        return apk[:, t, 0:L]
    def wqs(t):
        return apk[:, t, L:L + A]
    def wks(t):
        return apk[:, t, L + A:L + 2 * A]
    def wr(t):
        return rpk[:, t, 0:NREL * HS]
    def wro(t):
        return rpk[:, t, NREL * HS:NREL * HS + HS]
    def wsl(t):
        return rpk[:, t, NREL * HS + HS:NREL * HS + 2 * HS]
    def wn(t):
        return rpk[:, t, NREL * HS + 2 * HS:NREL * HS + 3 * HS]
    vv = spk[:, 0:1]
    maskt = spk[0:L, 1:1 + NREL * L].rearrange("i (r j) -> i r j", r=NREL)
    brg = spk[0:HS, 1 + NREL * L:2 + NREL * L]
    bgc = spk[0:HS, 2 + NREL * L:3 + NREL * L]

    # f32r (TF32-like) matmul operands: 4x PE rate on wide outputs, ~1e-3
    # matmul precision -- well inside the output tolerance here.
    f32r = mybir.dt.float32r
    # ---- Bahdanau attention: scores[i,j] = v . tanh(qT[:,i]+kT[:,j]) ----
    # qT/kT stay in PSUM; the broadcast add reads them there directly
    qT_ps = psum.tile([128, L, 1], dt, name=f"qT_ps{u}", tag="qt_ps")
    for t in range(2):
        nc.tensor.matmul(qT_ps[:, :, 0], wqs(t).bitcast(dt), xt(t).bitcast(dt),
                         start=(t == 0), stop=(t == 1))
    kT_ps = psum.tile([128, 1, L], dt, name=f"kT_ps{u}", tag="kt_ps")
    for t in range(2):
        nc.tensor.matmul(kT_ps[:, 0, :], wks(t).bitcast(dt), xt(t).bitcast(dt),
                         start=(t == 0), stop=(t == 1))
    kTs = pool.tile([128, 1, L], dt, name=f"kTs{u}", tag="kTs")
    nc.vector.tensor_copy(kTs[:, 0, :], kT_ps[:, 0, :])

    # broadcast add + tanh + v-matmul, in 2 halves so DVE/ACT/PE pipeline
    HL = L // 2  # 24 query rows per half
    bigT = pool.tile([128, L, L], dt, name=f"bigT{u}", tag="bigT")
    # tanh output + v are written as float32r so the score matmuls can run
    # the 4x-rate f32r PE path (producers must round to f32r per verifier)
    tanhT = pool.tile([128, L * L], f32r, name=f"tanhT{u}", tag="tanhT")
    vvr = pool.tile([128, 1], f32r, name=f"vvr{u}", tag="vvr")
    nc.vector.tensor_copy(vvr[:], vv)
    scores_ps = psum.tile([1, L * L], dt, name=f"scores_ps{u}", tag="attn_big")
    for h in range(2):
        isl = slice(h * HL, (h + 1) * HL)
        csl = slice(h * HL * L, (h + 1) * HL * L)
        nc.vector.tensor_tensor(
            bigT[:, isl, :],
            qT_ps[:, isl, :].broadcast_to([128, HL, L]),
            kTs[:].broadcast_to([128, HL, L]),
            op=mybir.AluOpType.add,
        )
        nc.scalar.activation(tanhT[:, csl],
                             bigT[:, isl, :].rearrange("p i j -> p (i j)"),
                             mybir.ActivationFunctionType.Tanh)
        # matmul outputs must stay inside one PSUM bank: chunk on global
        # 512-aligned boundaries; emit the chunks fully covered by the tanh
        # ready so far (half 0: [0:1024]; half 1: the rest).
        lo = 1024 if h == 1 else 0
        hi_lim = HL * L * (h + 1) if h == 0 else L * L
        for k in range(lo, hi_lim - 511 if h == 0 else hi_lim, 512):
            hi = min(k + 512, L * L)
            nc.tensor.matmul(scores_ps[:, k:hi], vvr[:], tanhT[:, k:hi],
                             start=True, stop=True)
    # PSUM -> SBUF [1, 2304]: single-partition copy, per-chunk, alternating
    # DVE/ACT so it pipelines behind the score matmuls
    scores_row = pool.tile([1, L * L], dt, name=f"scores_row{u}", tag="scores_row")
    for ci, k in enumerate(range(0, L * L, 512)):
        hi = min(k + 512, L * L)
        if ci % 2 == 0:
            nc.vector.tensor_copy(scores_row[:, k:hi], scores_ps[:, k:hi])
        else:
            nc.scalar.copy(scores_row[:, k:hi], scores_ps[:, k:hi])
    # reshape [1, 2304] -> [48, 48] has to cross partitions: DRAM bounce
    scores_dram = dram.tile([1, L * L], dt, name=f"scores_dram{u}", tag="scores_dram")
    nc.sync.dma_start(scores_dram[:], scores_row[:])
    scores = pool.tile([L, L], dt, name=f"scores{u}", tag="scores")
    sd_half = scores_dram[:].rearrange("p (a i j) -> a (p i) j", a=2, i=L // 2)
    nc.sync.dma_start(scores[0:L // 2, :], sd_half[0])
    nc.scalar.dma_start(scores[L // 2:L, :], sd_half[1])

    # ---- softmax over j (scores bounded by sum|v| ~ 9: no max pass) ----
    expS = pool.tile([L, L], dt, name=f"expS{u}", tag="expS")
    rowsum = pool.tile([L, 1], dt, name=f"rowsum{u}", tag="rowsum")
    nc.scalar.activation(expS[:], scores[:], mybir.ActivationFunctionType.Exp,
                         accum_out=rowsum[:])
    recip = pool.tile([L, 1], dt, name=f"recip{u}", tag="recip")
    nc.vector.reciprocal(recip[:], rowsum[:])
    # A_r[i,j] = (exp * 1/rowsum) * mask_r, all 8 relations in one op
    attnW = pool.tile([L, NREL, L], dt, name=f"attnW{u}", tag="attnW")
    NH = NREL // 2
    for a in range(2):
        rsl = slice(a * NH, (a + 1) * NH)
        nc.vector.scalar_tensor_tensor(
            attnW[:, rsl, :],
            expS[:].rearrange("i (o j) -> i o j", o=1).broadcast_to([L, NH, L]),
            recip[:],
            maskt[:, rsl, :],
            op0=mybir.AluOpType.mult,
            op1=mybir.AluOpType.mult,
        )

    # ---- RGCN (h-slice): Yall[i, r*32+h'] = (x @ W_r[:, hsl]) ----
    yall_ps = psum.tile([L, NREL * HS], dt, name=f"yall_ps{u}", tag="mm_ps")
    for t in range(2):
        nc.tensor.matmul(yall_ps[:], xt(t).bitcast(dt), wr(t).bitcast(dt),
                         start=(t == 0), stop=(t == 1))
    yall = pool.tile([L, NREL * HS], dt, name=f"yall{u}", tag="yall")
    nc.vector.tensor_copy(yall[:], yall_ps[:])

    # hT_slice[h', j] = sum_r sum_i Y_r[i,h'] A_r[i,j] + (x@W_root)^T + b
    h_ps = psum.tile([HS, L], dt, name=f"h_ps{u}", tag="mm_ps")
    for r in range(NREL):
        nc.tensor.matmul(h_ps[:], yall[:, r * HS:(r + 1) * HS], attnW[:, r, :],
                         start=(r == 0), stop=False)
    for t in range(2):
        nc.tensor.matmul(h_ps[:], wro(t).bitcast(dt), xt(t).bitcast(dt),
                         start=False, stop=(t == 1))
    hTs = pool.tile([HS, L], dt, name=f"hTs{u}", tag="hTs")
    nc.vector.tensor_scalar_add(hTs[:], h_ps[:], brg)

    # ---- AllGather h-slices -> full hT [256, 48] on every core ----
    ag_in = dram.tile([HS, L], dt, name=f"ag_in{u}", tag="ag_in")
    ag_out = dram.tile([H, L], dt, name=f"ag_out{u}", tag="ag_out")
    nc.sync.dma_start(ag_in[:], hTs[:])
    if collective:
        nc.gpsimd.collective_compute(
            "AllGather",
            mybir.AluOpType.bypass,
            replica_groups=[list(range(n_cores))],
            ins=[ag_in.opt()],
            outs=[ag_out.opt()],
        )
    else:
        # single-core stand-in for TimelineSim: replicate the slice 8x
        agw = ag_out[:].rearrange("(c p) f -> c p f", p=HS)
        for c in range(N_CORES):
            nc.sync.dma_start(agw[c], ag_in[:])
    hfull = pool.tile([128, 2, L], dt, name=f"hfull{u}", tag="hfull")
    agv = ag_out[:].rearrange("(t p) f -> t p f", p=128)
    nc.sync.dma_start(hfull[:, 0, :], agv[0])
    nc.scalar.dma_start(hfull[:, 1, :], agv[1])

    # ---- GraphConv (g-slice): out^T = W_self^T hT + (W_nbr^T s + b) ----
    sT = pool.tile([128, 2], dt, name=f"sT{u}", tag="sT")
    for t in range(2):
        nc.vector.reduce_sum(sT[:, t:t + 1], hfull[:, t, :],
                             axis=mybir.AxisListType.X)
    nb_ps = psum.tile([HS, 1], dt, name=f"nb_ps{u}", tag="mm_ps")
    for t in range(2):
        nc.tensor.matmul(nb_ps[:], wn(t).bitcast(dt), sT[:, t:t + 1],
                         start=(t == 0), stop=(t == 1))
    nbs = pool.tile([HS, 1], dt, name=f"nbs{u}", tag="nbs")
    nc.vector.tensor_scalar_add(nbs[:], nb_ps[:], bgc)

    out_ps = psum.tile([HS, L], dt, name=f"out_ps{u}", tag="mm_ps")
    for t in range(2):
        nc.tensor.matmul(out_ps[:], wsl(t).bitcast(dt), hfull[:, t, :],
                         start=(t == 0), stop=(t == 1))
    outs = pool.tile([HS, L], dt, name=f"outs{u}", tag="outs")
    nc.vector.tensor_scalar_add(outs[:], out_ps[:], nbs)
    nc.sync.dma_start(d["yout"].ap(), outs[:])


def build_program(n_cores=N_CORES, collective=True, repeat=1):
    """Build + schedule + compile the Bass program."""
    import concourse.bacc as bacc
    import concourse.mybir as mybir
    import concourse.tile as tile

    dt = mybir.dt.float32
    nc = bacc.Bacc("TRN2", debug=False, num_devices=n_cores)

    d = {}
    d["apack"] = nc.dram_tensor("apack", [2, 128, 304], mybir.dt.float32r,
                            kind="ExternalInput")
    d["rpack"] = nc.dram_tensor("rpack", [2, 128, 352], mybir.dt.float32r,
                            kind="ExternalInput")
    d["spack"] = nc.dram_tensor("spack", [128, 387], dt, kind="ExternalInput")
    d["yout"] = nc.dram_tensor("yout", [HS, L], dt, kind="ExternalOutput")

    with tile.TileContext(nc) as tc:
        with (
            tc.tile_pool(name="sbuf", bufs=1) as pool,
            tc.tile_pool(name="psum", bufs=1, space="PSUM") as psum,
            tc.tile_pool(name="dram", bufs=1, space="DRAM") as dram,
        ):
            for rep in range(repeat):
                _emit_body(nc, mybir, pool, psum, dram, d, rep, collective, n_cores)

    nc.compile()
    return nc


def _prepare_in_maps(global_features, speaker, Wq, Wk, v, W_rel, W_root, b_rgcn,
                     W_nbr, W_self, b_gcn):
    """Host-side routing: pick the <=8 live relation slices, build masks, pack
    per-core shards (h-slice of RGCN weights, g-slice of GraphConv weights)."""
    f32 = np.float32
    x = np.ascontiguousarray(global_features, dtype=f32)
    sp = np.asarray(speaker).astype(np.int64)
    n = L

    ii, jj = np.meshgrid(np.arange(n), np.arange(n), indexing="ij")
    direction = (ii >= jj).astype(np.int64)
    et = 2 * (sp[ii] * n + sp[jj]) + direction  # [48, 48] edge-type grid

    rel_ids = np.unique(et)
    assert len(rel_ids) <= NREL, f"{len(rel_ids)} live relations > {NREL}"
    masks = np.zeros((NREL, n, n), dtype=f32)
    rel_pad = np.full(NREL, rel_ids[0], dtype=np.int64)
    for s, rid in enumerate(rel_ids):
        masks[s] = (et == rid)
        rel_pad[s] = rid
    # padded slots keep zero masks -> contribute nothing

    W_used = np.ascontiguousarray(np.asarray(W_rel)[rel_pad], dtype=f32)  # [8,256,256]

    xt = np.ascontiguousarray(x.T).reshape(2, 128, L)
    wq = np.ascontiguousarray(Wq, dtype=f32).reshape(2, 128, A)
    wk = np.ascontiguousarray(Wk, dtype=f32).reshape(2, 128, A)
    maskw = np.ascontiguousarray(masks.transpose(1, 0, 2)).reshape(L, NREL * L)
    apack = np.ascontiguousarray(np.concatenate([xt, wq, wk], axis=2))
    W_root = np.asarray(W_root, dtype=f32)
    W_self = np.asarray(W_self, dtype=f32)
    W_nbr = np.asarray(W_nbr, dtype=f32)
    b_rgcn = np.asarray(b_rgcn, dtype=f32)
    b_gcn = np.asarray(b_gcn, dtype=f32)

    in_maps = []
    for c in range(N_CORES):
        sl = slice(c * HS, (c + 1) * HS)
        wrel_c = np.ascontiguousarray(
            W_used[:, :, sl].transpose(1, 0, 2)).reshape(2, 128, NREL * HS)
        rpack = np.ascontiguousarray(np.concatenate([
            wrel_c,
            W_root[:, sl].reshape(2, 128, HS),
            W_self[:, sl].reshape(2, 128, HS),
            W_nbr[:, sl].reshape(2, 128, HS),
        ], axis=2))
        spack = np.zeros((128, 3 + NREL * L), dtype=f32)
        spack[:, 0] = np.ascontiguousarray(v, dtype=f32).reshape(128)
        spack[0:L, 1:1 + NREL * L] = maskw
        spack[0:HS, 1 + NREL * L] = b_rgcn[sl]
        spack[0:HS, 2 + NREL * L] = b_gcn[sl]
        in_maps.append({"apack": apack, "rpack": rpack, "spack": spack})
    return in_maps


def kernel(global_features, speaker, Wq, Wk, v, W_rel, W_root, b_rgcn,
           W_nbr, W_self, b_gcn):
    global _compiled
    from concourse.bass_utils import run_bass_kernel_spmd

    if _compiled is None:
        _compiled = build_program()
    nc = _compiled

    in_maps = _prepare_in_maps(global_features, speaker, Wq, Wk, v, W_rel,
                               W_root, b_rgcn, W_nbr, W_self, b_gcn)
    res = run_bass_kernel_spmd(nc, in_maps, core_ids=list(range(N_CORES)))
    outT = np.concatenate([res.results[c]["yout"] for c in range(N_CORES)], axis=0)
    return np.ascontiguousarray(outT.T)
